# revision 1
# baseline (speedup 1.0000x reference)
"""BiAttention (BiDAF-style) Trainium2 kernel, bf16-I/O version.

Full f32 inputs -> host-cast context/query to bf16 -> shard batch dim over 8
NeuronCores (4 batches each) -> SPMD Bass/Tile kernel writing G cols 1-3 in
bf16 -> host-cast back to f32.  bf16 end-to-end error is ~2e-3 (gate is 2e-2)
and halves the HBM traffic of every major tensor; G column 0 is bit-exactly
the bf16 context, so the host assembles it directly and the device never
round-trips it.  Per core: 4 MiB context in, 0.5 MiB query in, 12 MiB G out
-> ~49 us DMA busy at the 360 GB/s the device serializes DMA transfers at.

Math (per batch), masks are exact {0,1}:
  Rp[d,j]  = bf16(qT[d,j]*w_cq[d] + w_c[d])
  sq[j]    = sum_d q[j,d] w_q[d]
  lng[j]   = sq[j] + ln(qm[j]+1e-38)          (kills masked j inside the exp)
  enT[j,c] = exp(sum_d Rp[d,j] cT[d,c] + lng[j])   (bias folded into ACT exp)
  den[c]   = sum_j enT[j,c]                   (ones-column matmuls, ap=1)
  c2q      = (enT^T @ q) / den
  mx[c]    = max_j enT[j,c]                   (DVE jc-premerge + gpsimd
                                               partition-axis reduce)
  E2[c]    = cm[c] * mx[c]
  q2c      = (E2 @ c) / sum_c E2
  G        = [c, c2q, c*c2q, c*q2c]

Layout notes: enT is computed transposed ([j,c]) so it feeds the c2q matmuls
as lhsT directly -- no en transposes and no bias matmuls.  All "transposes"
are plain matmuls against a bf16 identity writing f32 psum (walrus rejects
non-f32 matmult psum outputs).  The emission is software-pipelined two deep:
stage A(s) = cT transposes/copies + S0T + exp, stage B(s) = everything that
consumes exp(s), emitted after A(s+1) so PE works on quad s+1 while ACT's exp
of quad s is in flight.  All loads and stores ride the compute-free SP queue
(stores staggered one slot behind their producers); elementwise work is
balanced across ACT/DVE/Pool per-slot, with a norm chunk moving to ACT on
alternating slots.
"""

import numpy as np
import ml_dtypes

import bass_rust
import concourse.bass as bass
import concourse.mybir as mybir
from concourse.tile import TileContext
from concourse.bass_utils import run_bass_kernel_spmd
from concourse.masks import make_identity

F32 = mybir.dt.float32
BF16 = mybir.dt.bfloat16
AF = mybir.ActivationFunctionType
OP = mybir.AluOpType
AX = mybir.AxisListType

N_CORES = 8
B, C_L, Q_L, D2 = 32, 2048, 256, 256
BPC = B // N_CORES          # batches per core
NQ = C_L // 512             # context quads per batch (quad = 4x128 rows)
G_W = 4 * D2
SETUP_SLOTS = (1, 5, 9)


def _spill_excess_waits(nc, max_waits: int = 1) -> int:
    """The installed walrus rejects >1 sync wait per instruction. Hoist excess
    waits onto same-engine InstNoOp carriers inserted just before."""
    n = 0
    uid = 0
    for f in nc.m.functions:
        for bb in f.blocks:
            out = []
            changed = False
            for inst in bb.instructions:
                si = inst.sync_info
                waits = list(si.on_wait) if si is not None and si.on_wait else []
                if len(waits) > max_waits:
                    head, tail = waits[:-max_waits], waits[-max_waits:]
                    for i in range(0, len(head), max_waits):
                        out.append(
                            mybir.InstNoOp(
                                name=f"I-wspill-{bb.name}-{uid}",
                                engine=inst.engine,
                                ins=[],
                                outs=[],
                                sync_info=bass_rust.SyncInfo(
                                    on_wait=head[i : i + max_waits], on_update=[]
                                ),
                            )
                        )
                        uid += 1
                        n += 1
                    si.on_wait = tail
                    changed = True
                out.append(inst)
            if changed:
                bb.instructions = out
    return n


def build_bass():
    nc = bass.Bass()
    ctx_h = nc.declare_dram_parameter("context", [BPC, C_L, D2], BF16, isOutput=False)
    cm_h = nc.declare_dram_parameter("context_mask", [BPC, C_L], F32, isOutput=False)
    q_h = nc.declare_dram_parameter("query", [BPC, Q_L, D2], BF16, isOutput=False)
    qm_h = nc.declare_dram_parameter("query_mask", [BPC, Q_L], F32, isOutput=False)
    w_h = nc.declare_dram_parameter("W", [3 * D2], F32, isOutput=False)
    # G columns 1-3 only: column 0 is bit-exactly the bf16 context, which the
    # host already holds -- no reason to round-trip it through the device.
    g_h = nc.declare_dram_parameter("G", [BPC, C_L, 3 * D2], BF16, isOutput=True)

    with TileContext(nc) as tc:
        with (
            tc.tile_pool(name="const", bufs=1) as cpool,
            tc.tile_pool(name="ld", bufs=1) as lpool,
            tc.tile_pool(name="ctx", bufs=BPC * NQ) as xpool,
            tc.tile_pool(name="wen", bufs=4) as wen,
            tc.tile_pool(name="wcomb", bufs=4) as wcomb,
            tc.tile_pool(name="wsm", bufs=2) as wsm,
            tc.tile_pool(name="ps_ct", bufs=2, space="PSUM") as ps_ct,
            tc.tile_pool(name="ps_en", bufs=2, space="PSUM") as ps_en,
            tc.tile_pool(name="ps_cq", bufs=2, space="PSUM") as ps_cq,
            tc.tile_pool(name="ps_den", bufs=1, space="PSUM") as ps_den,
            tc.tile_pool(name="ps_u", bufs=1, space="PSUM") as ps_u,
        ):
            # ---------------- loads (SP queue: pure prefetch) ----------------
            # batch 0's query first so its setup chain starts ASAP
            q_sb = lpool.tile([128, BPC * 2 * D2], BF16)

            def load_q(b):
                nc.sync.dma_start(
                    out=q_sb[:, b * 2 * D2 : (b + 1) * 2 * D2].rearrange(
                        "p (jc d) -> p jc d", jc=2
                    ),
                    in_=q_h[b].rearrange("(jc p) d -> p jc d", p=128),
                )

            load_q(0)
            w6 = lpool.tile([128, 6], F32)
            nc.sync.dma_start(out=w6[:], in_=w_h.rearrange("(a p) -> p a", p=128))
            qm_sb = lpool.tile([128, 2 * BPC], F32)
            nc.sync.dma_start(
                out=qm_sb[:].rearrange("p (b jc) -> p b jc", b=BPC),
                in_=qm_h.rearrange("b (jc p) -> p b jc", p=128),
            )
            LCM = lpool.tile([16, BPC * 128], F32)
            nc.sync.dma_start(
                out=LCM[:].rearrange("i (b p) -> i b p", b=BPC),
                in_=cm_h.rearrange("b (i p) -> i b p", p=128),
            )
            c_quads = {}

            def load_quad(b, p):
                cq = xpool.tile([128, 1024], BF16, tag="c", name=f"c{b}{p}")
                nc.sync.dma_start(
                    out=cq[:].rearrange("p (t d) -> p t d", t=4),
                    in_=ctx_h[b, p * 512 : (p + 1) * 512, :].rearrange(
                        "(t p) d -> p t d", p=128
                    ),
                )
                c_quads[(b, p)] = cq

            # first two batches up front; the rest stream into the store
            # phase (6 slots ahead of use) so stores start flowing earlier
            load_quad(0, 0)
            for b in range(1, BPC):
                load_q(b)
            for p in range(1, NQ):
                load_quad(0, p)
            for p in range(NQ):
                load_quad(1, p)

            # ---------------- constants ----------------
            ident = cpool.tile([128, 128], F32)
            make_identity(nc, ident[:])
            identb = cpool.tile([128, 128], BF16)
            nc.vector.tensor_copy(identb[:], ident[:])
            ones_row_b = cpool.tile([1, 128], BF16)
            nc.vector.memset(ones_row_b[:], 1.0)
            ones_col_b = cpool.tile([128, 1], BF16)
            nc.vector.memset(ones_col_b[:], 1.0)
            ones_col_f = cpool.tile([128, 1], F32)
            nc.vector.memset(ones_col_f[:], 1.0)
            eps_col = cpool.tile([128, 1], F32)
            nc.vector.memset(eps_col[:], 1e-38)
            wqb = cpool.tile([128, 2], BF16)
            nc.vector.tensor_copy(wqb[:], w6[:, 2:4])

            # ---------------- per-batch setup (all batches up front) --------
            qT_sb = lpool.tile([128, BPC * 512], BF16)
            Rp_sb = lpool.tile([128, BPC * 512], BF16)
            lng_sb = lpool.tile([128, 2 * BPC], F32)
            lnqm_sb = lpool.tile([128, 2 * BPC], F32)
            CM_sb = lpool.tile([128, BPC * 4 * NQ], BF16)
            E2_sb = lpool.tile([128, BPC * 4 * NQ], BF16)

            def setup_batch(b):
                # qT: [j,d] -> [d,j] per (dc, jc) block.  All "transposes" are
                # plain matmuls against a bf16 identity: out = lhsT^T in f32
                # psum (walrus rejects non-f32 matmult psum outputs).
                qt_ps = ps_ct.tile([128, 512], F32, tag="ct", name=f"qt{b}")
                for dc in range(2):
                    for jc in range(2):
                        nc.tensor.matmul(
                            qt_ps[:, dc * 256 + jc * 128 : dc * 256 + (jc + 1) * 128],
                            q_sb[
                                :,
                                (b * 2 + jc) * D2 + dc * 128 : (b * 2 + jc) * D2
                                + (dc + 1) * 128,
                            ],
                            identb[:],
                            start=True,
                            stop=True,
                        )
                nc.scalar.copy(qT_sb[:, b * 512 : (b + 1) * 512], qt_ps[:])
                # Rp = qT*w_cq + w_c (bf16), sq = w_q . qT (psum f32)
                sm = ps_den.tile([128, 16], F32, tag="den", name=f"sq{b}")
                for dc in range(2):
                    nc.vector.tensor_scalar(
                        out=Rp_sb[:, b * 512 + dc * 256 : b * 512 + (dc + 1) * 256],
                        in0=qT_sb[:, b * 512 + dc * 256 : b * 512 + (dc + 1) * 256],
                        scalar1=w6[:, 4 + dc : 5 + dc],
                        scalar2=w6[:, 0 + dc : 1 + dc],
                        op0=OP.mult,
                        op1=OP.add,
                    )
                for jc in range(2):
                    for dc in range(2):
                        nc.tensor.matmul(
                            sm[:, 6 + jc : 7 + jc],
                            qT_sb[
                                :,
                                b * 512 + dc * 256 + jc * 128 : b * 512
                                + dc * 256
                                + (jc + 1) * 128,
                            ],
                            wqb[:, dc : dc + 1],
                            start=(dc == 0),
                            stop=(dc == 1),
                        )
                if b == 0:
                    nc.scalar.activation(
                        lnqm_sb[:], qm_sb[:], AF.Ln, bias=eps_col[:]
                    )
                nc.vector.tensor_add(
                    lng_sb[:, b * 2 : b * 2 + 2], sm[:, 6:8], lnqm_sb[:, b * 2 : b * 2 + 2]
                )
                # CM: [16,128] slab -> [128,16] via matmul against identity
                cmt_ps = ps_ct.tile([128, 512], F32, tag="ct", name=f"cmt{b}")
                nc.tensor.matmul(
                    cmt_ps[:, 0:16],
                    LCM[:, b * 128 : (b + 1) * 128],
                    ident[0:16, 0:16],
                    start=True,
                    stop=True,
                )
                nc.vector.tensor_copy(
                    CM_sb[:, b * 16 : (b + 1) * 16], cmt_ps[:, 0:16]
                )

            setup_batch(0)

            # ---------------- main loops (2-stage software pipeline) --------
            # Stage A(s) = transposes + S0T matmuls + exp; stage B(s) = all
            # work that consumes exp(s).  Emitting A(s+1) before B(s) keeps
            # the PE busy on the next quad while the ACT exp of the current
            # one is still in flight.  Stores are flushed at the start of the
            # following B so their queue waits are satisfied at dispatch.
            S_TOT = BPC * NQ
            pend_stores = []
            u2c_ps = {}
            Q2C_of = {}

            def flush_stores():
                for fn in pend_stores:
                    fn()
                pend_stores.clear()

            def phase2a(b, den):
                # q2c row for batch b.  The u2c psum is copied to sbuf in ONE
                # hop after its stop-matmul so the bank frees immediately (the
                # next batch's u2c accumulation is on the PE critical path);
                # the z2 scalar goes into the quad's den tile (cols 12:13) and
                # the normalization happens off the critical path.
                u_row = wsm.tile([1, D2], F32, tag="urow", name="urow")
                nc.vector.tensor_copy(u_row[:], u2c_ps[b][0:1, 0:256])
                z2 = wsm.tile([128, 1], F32, tag="z2", name="z2")
                nc.vector.reduce_sum(
                    z2[:], E2_sb[:, b * 16 : (b + 1) * 16], axis=AX.X
                )
                nc.tensor.matmul(
                    den[0:1, 12:13], z2[:], ones_col_f[:], start=True, stop=True
                )
                rz = wsm.tile([1, 1], F32, tag="rz", name="rz")
                nc.vector.reciprocal(rz[:], den[0:1, 12:13])
                q2c_row = wsm.tile([1, D2], BF16, tag="q2cr", name="q2cr")
                nc.vector.tensor_scalar_mul(q2c_row[:], u_row[:], rz[:])
                return q2c_row

            def q2c_broadcast(q2c_row):
                qb_ps = ps_cq.tile([128, 512], F32, tag="cq", name="qbps")
                nc.tensor.matmul(
                    qb_ps[:, 0:256], ones_row_b[:], q2c_row[:], start=True, stop=True
                )
                Q2C = wsm.tile([128, D2], BF16, tag="Q2C", name="Q2C")
                nc.scalar.copy(Q2C[:], qb_ps[:, 0:256])
                return Q2C

            def stageA_t(b, p):
                c_quad = c_quads[(b, p)]
                # cT: 8 identity-matmul transposes -> two f32 psum banks ->
                # sbuf bf16 (ACT)
                cT_sb = wsm.tile([128, 1024], BF16, tag="cT", name="cT")
                for dc in range(2):
                    ct_ps = ps_ct.tile([128, 512], F32, tag="ct", name="ct")
                    for t in range(4):
                        nc.tensor.matmul(
                            ct_ps[:, t * 128 : (t + 1) * 128],
                            c_quad[:, t * 256 + dc * 128 : t * 256 + (dc + 1) * 128],
                            identb[:],
                            start=True,
                            stop=True,
                        )
                    nc.scalar.copy(cT_sb[:, dc * 512 : (dc + 1) * 512], ct_ps[:])
                return cT_sb

            def stageA_mm(b, p, cT_sb):
                # S0T -> exp (bias = lng per-partition) -> enT [j, c]
                en_sb = wen.tile([128, 1024], BF16, tag="en", name="en")
                for jc in range(2):
                    en_ps = ps_en.tile([128, 512], F32, tag="en", name="enps")
                    for dc in range(2):
                        nc.tensor.matmul(
                            en_ps[:],
                            Rp_sb[
                                :,
                                b * 512 + dc * 256 + jc * 128 : b * 512
                                + dc * 256
                                + (jc + 1) * 128,
                            ],
                            cT_sb[:, dc * 512 : (dc + 1) * 512],
                            start=(dc == 0),
                            stop=(dc == 1),
                        )
                    nc.scalar.activation(
                        en_sb[:, jc * 512 : (jc + 1) * 512],
                        en_ps[:],
                        AF.Exp,
                        bias=lng_sb[:, b * 2 + jc : b * 2 + jc + 1],
                    )

                return en_sb

            def stageB_den(b, p, en_sb):
                # denominators (ones matmuls, ap=1): emitted between the next
                # quad's transposes and its S0T so DVE's rden unblocks early
                den = ps_den.tile([128, 16], F32, tag="den", name="den")
                for ch in range(4):
                    for jc in range(2):
                        nc.tensor.matmul(
                            den[:, ch : ch + 1],
                            en_sb[:, jc * 512 + ch * 128 : jc * 512 + (ch + 1) * 128],
                            ones_col_b[:],
                            start=(jc == 0),
                            stop=(jc == 1),
                        )
                return (den,)

            def stageB(b, p, en_sb, den):
                s = b * NQ + p
                if 2 <= s <= 9:
                    load_quad(*divmod(s + 6, NQ))
                flush_stores()
                if p == 0 and b > 0:
                    Q2C_of[b - 1] = q2c_broadcast(q2c_row_of[b - 1])
                c_quad = c_quads[(b, p)]

                # prev batch's cq2c first in the DVE stream: its input (Q2C)
                # is long since ready, so it fills DVE while den completes
                if b > 0:
                    cq2c = wcomb.tile([128, 1024], BF16, tag="cqc", name="cqc")
                    nc.vector.tensor_mul(
                        cq2c[:].rearrange("p (t d) -> p t d", t=4),
                        c_quads[(b - 1, p)][:].rearrange("p (t d) -> p t d", t=4),
                        Q2C_of[b - 1][:]
                        .rearrange("p (o d) -> p o d", o=1)
                        .broadcast_to([128, 4, D2]),
                    )

                    def store_col3(bb=b - 1, pp=p, src=cq2c):
                        nc.sync.dma_start(
                            out=g_h[
                                bb, pp * 512 : (pp + 1) * 512, 2 * D2 : 3 * D2
                            ].rearrange("(t p) d -> p t d", p=128),
                            in_=src[:].rearrange("p (t d) -> p t d", t=4),
                        )

                    store_col3()

                # c2q matmuls + normalization (DVE 3D broadcast-mults)
                cq_ps = []
                for h in range(2):
                    cp = ps_cq.tile([128, 512], F32, tag="cq", name="cqps")
                    for x in range(2):
                        ch = 2 * h + x
                        for jc in range(2):
                            nc.tensor.matmul(
                                cp[:, x * 256 : (x + 1) * 256],
                                en_sb[
                                    :,
                                    jc * 512 + ch * 128 : jc * 512 + (ch + 1) * 128,
                                ],
                                q_sb[:, (b * 2 + jc) * D2 : (b * 2 + jc + 1) * D2],
                                start=(jc == 0),
                                stop=(jc == 1),
                            )
                    cq_ps.append(cp)
                rden = wsm.tile([128, 4], F32, tag="rden", name="rden")
                nc.vector.reciprocal(rden[:], den[:, 0:4])
                comb = wcomb.tile([128, 2048], BF16, tag="comb", name="comb")
                if s % 2 == 1:
                    nc.scalar.activation(
                        comb[:, 0:256],
                        cq_ps[0][:, 0:256],
                        AF.Identity,
                        scale=rden[:, 0:1],
                    )
                    nc.vector.tensor_mul(
                        comb[:, 512 : 512 + 256],
                        cq_ps[0][:, 256:512],
                        rden[:, 1:2].broadcast_to([128, 256]),
                    )
                    hs = (1,)
                else:
                    hs = (0, 1)
                for h in hs:
                    nc.vector.tensor_mul(
                        comb[:, h * 1024 : (h + 1) * 1024]
                        .rearrange("p (x q d) -> p x q d", x=2, q=2)[:, :, 0, :],
                        cq_ps[h][:].rearrange("p (x d) -> p x d", x=2),
                        rden[:, 2 * h : 2 * h + 2]
                        .rearrange("p (x o) -> p x o", o=1)
                        .broadcast_to([128, 2, 256]),
                    )

                # mx over j: DVE pre-merges the jc halves (so the gpsimd
                # partition reduce only sees 512 cols), then 4 tiny identity-
                # matmul transposes back to column layout (f32 psum cols 8..11)
                enM = wsm.tile([128, 512], BF16, tag="enM", name="enM")
                nc.vector.tensor_tensor(
                    out=enM[:], in0=en_sb[:, 0:512], in1=en_sb[:, 512:1024],
                    op=OP.max,
                )
                m1 = wsm.tile([1, 512], BF16, tag="m1", name="m1")
                nc.gpsimd.tensor_reduce(
                    out=m1[:], in_=enM[:], axis=AX.C, op=OP.max
                )
                for t in range(4):
                    nc.tensor.matmul(
                        den[:, 8 + t : 9 + t],
                        m1[0:1, t * 128 : (t + 1) * 128],
                        identb[0:1, 0:1],
                        start=True,
                        stop=True,
                    )
                nc.vector.tensor_mul(
                    E2_sb[:, b * 16 + 4 * p : b * 16 + 4 * p + 4],
                    den[:, 8:12],
                    CM_sb[:, b * 16 + 4 * p : b * 16 + 4 * p + 4],
                )

                # u2c accumulation across the batch
                if p == 0:
                    u2c_ps[b] = ps_u.tile([1, 256], F32, tag="u", name="u")
                for t in range(4):
                    nc.tensor.matmul(
                        u2c_ps[b][0:1, 0:256],
                        E2_sb[:, b * 16 + 4 * p + t : b * 16 + 4 * p + t + 1],
                        c_quad[:, t * 256 : (t + 1) * 256],
                        start=(p == 0 and t == 0),
                        stop=(p == NQ - 1 and t == 3),
                    )

                # cc2q into the combined tile (strided interleave); half on
                # DVE, half on Pool
                for h, eng in ((0, nc.vector), (1, nc.gpsimd)):
                    hv = comb[:, h * 1024 : (h + 1) * 1024].rearrange(
                        "p (t x d) -> p t x d", t=2, x=2
                    )
                    eng.tensor_mul(
                        hv[:, :, 1, :],
                        c_quad[:, h * 512 : (h + 1) * 512].rearrange(
                            "p (t d) -> p t d", t=2
                        ),
                        hv[:, :, 0, :],
                    )

                def store_col12(bb=b, pp=p, src=comb):
                    nc.sync.dma_start(
                        out=g_h[bb, pp * 512 : (pp + 1) * 512, 0 : 2 * D2].rearrange(
                            "(t p) d -> p t d", p=128
                        ),
                        in_=src[:].rearrange("p (t d) -> p t d", t=4),
                    )

                store_col12()

                # start the q2c chain as soon as this batch's E2/u2c complete
                if p == NQ - 1:
                    q2c_row_of[b] = phase2a(b, den)

            q2c_row_of = {}
            stash = {}
            for s in range(S_TOT):
                b, p = divmod(s, NQ)
                if s in SETUP_SLOTS:
                    setup_batch(SETUP_SLOTS.index(s) + 1)
                cT_sb = stageA_t(b, p)
                if s > 0:
                    pb, pp, pen = stash[s - 1]
                    pdrc = stageB_den(pb, pp, pen)
                en_sb = stageA_mm(b, p, cT_sb)
                if s > 0:
                    stageB(pb, pp, pen, *pdrc)
                stash[s] = (b, p, en_sb)
            lb, lp, len_sb = stash[S_TOT - 1]
            ldrc = stageB_den(lb, lp, len_sb)
            stageB(lb, lp, len_sb, *ldrc)

            # ---------------- tail: batch BPC-1 col3 ----------------
            flush_stores()
            Q2C_of[BPC - 1] = q2c_broadcast(q2c_row_of[BPC - 1])
            for p in range(NQ):
                cq2c = wcomb.tile([128, 1024], BF16, tag="cqc", name="cqc")
                eng = nc.vector if p % 2 == 0 else nc.gpsimd
                eng.tensor_mul(
                    cq2c[:].rearrange("p (t d) -> p t d", t=4),
                    c_quads[(BPC - 1, p)][:].rearrange("p (t d) -> p t d", t=4),
                    Q2C_of[BPC - 1][:]
                    .rearrange("p (o d) -> p o d", o=1)
                    .broadcast_to([128, 4, D2]),
                )
                nc.sync.dma_start(
                    out=g_h[
                        BPC - 1, p * 512 : (p + 1) * 512, 2 * D2 : 3 * D2
                    ].rearrange("(t p) d -> p t d", p=128),
                    in_=cq2c[:].rearrange("p (t d) -> p t d", t=4),
                )

    _spill_excess_waits(nc)
    return nc


_NC_CACHE = None


def _get_nc():
    global _NC_CACHE
    if _NC_CACHE is None:
        _NC_CACHE = build_bass()
    return _NC_CACHE


def kernel(**inputs) -> np.ndarray:
    bf16 = ml_dtypes.bfloat16
    ctx = np.ascontiguousarray(np.asarray(inputs["context"], dtype=np.float32)).astype(bf16)
    cm = np.ascontiguousarray(np.asarray(inputs["context_mask"], dtype=np.float32))
    q = np.ascontiguousarray(np.asarray(inputs["query"], dtype=np.float32)).astype(bf16)
    qm = np.ascontiguousarray(np.asarray(inputs["query_mask"], dtype=np.float32))
    w = np.ascontiguousarray(np.asarray(inputs["W"], dtype=np.float32))

    in_maps = []
    for core in range(N_CORES):
        lo, hi = core * BPC, (core + 1) * BPC
        in_maps.append(
            {
                "context": ctx[lo:hi],
                "context_mask": cm[lo:hi],
                "query": q[lo:hi],
                "query_mask": qm[lo:hi],
                "W": w,
            }
        )

    nc = _get_nc()
    res = run_bass_kernel_spmd(nc, in_maps, list(range(N_CORES)))
    out = np.empty((B, C_L, G_W), dtype=np.float32)
    # col0 of G is exactly the bf16 context (bit-identical to what the device
    # would have copied back); cols 1-3 come from the device.
    out[:, :, 0:D2] = ctx.astype(np.float32)
    for i in range(N_CORES):
        out[i * BPC : (i + 1) * BPC, :, D2:] = np.asarray(
            res.results[i]["G"]
        ).astype(np.float32)
    return out



# revision 3
# speedup vs baseline: 1.4699x; 1.4699x over previous
"""BiAttention (BiDAF-style) Trainium2 kernel, v2: device computes only the
irreducible attention core; everything reconstructible from small device
outputs is assembled on the host.

G = [c, c2q, c*c2q, c*q2c].  The host already holds c, so the device only
needs to produce
  c2q[b,c,:]  (the C2Q attention-weighted query rows, bf16, 4 MiB/core) and
  mx[b,c]     (max_j of the masked exp similarities, f32, 32 KB/core),
from which the host derives s_max = ln(mx), the Q2C softmax, q2c, and the
three elementwise output columns in f32 (col0 is exactly c).

Device math per batch (masks exact {0,1}):
  Rp[d,j]  = bf16(qT[d,j]*w_cq[d] + w_c[d])        (host-computed)
  lng[j]   = q[j]·w_q + ln(qm[j]+1e-38)            (host-computed)
  enT[j,c] = exp(sum_d Rp[d,j] ctxT[d,c] + lng[j]) (PE + ACT exp bias)
  den[c]   = sum_j enT[j,c]                        (ones-column matmuls)
  c2q      = (enT^T @ q) * (1/den)                 (PE + DVE bcast-mult)
  mx[c]    = max_j enT[j,c]                        (DVE jc-premerge + gpsimd
                                                    partition-axis reduce)

The context arrives HOST-PRE-TRANSPOSED as ctxT[b, d, c], so the PE does no
transposes at all: every matmul contracts over d (S) or j (c2q/den) with both
operands already partition-major in the contraction dim.  Per 512-column slot
the PE streams 4096 output rows (S 2048 + c2q 2048 + 8 one-col den rows);
DMA moves 9.0 MiB/core; ACT does only the exps; DVE does reciprocal +
normalize-copy + jc-premerge; gpsimd only the partition-axis max.

Emission is software-pipelined one slot deep: A(s) = S matmuls + exps,
B(s-1) = den/c2q matmuls and the vector tail of the previous slot, so the PE
works on slot s while ACT exps of slot s and the DVE/Pool tail of s-1 drain.
"""

import numpy as np
import ml_dtypes

import bass_rust
import concourse.bass as bass
import concourse.mybir as mybir
from concourse.tile import TileContext
from concourse.bass_utils import run_bass_kernel_spmd

F32 = mybir.dt.float32
BF16 = mybir.dt.bfloat16
AF = mybir.ActivationFunctionType
OP = mybir.AluOpType
AX = mybir.AxisListType

N_CORES = 8
B, C_L, Q_L, D2 = 32, 2048, 256, 256
BPC = B // N_CORES          # batches per core
NSLOT = BPC * 4             # 512-column slots per core
EPS = 1e-13


def _spill_excess_waits(nc, max_waits: int = 1) -> int:
    """The installed walrus rejects >1 sync wait per instruction. Hoist excess
    waits onto same-engine InstNoOp carriers inserted just before."""
    n = 0
    uid = 0
    for f in nc.m.functions:
        for bb in f.blocks:
            out = []
            changed = False
            for inst in bb.instructions:
                si = inst.sync_info
                waits = list(si.on_wait) if si is not None and si.on_wait else []
                if len(waits) > max_waits:
                    head, tail = waits[:-max_waits], waits[-max_waits:]
                    for i in range(0, len(head), max_waits):
                        out.append(
                            mybir.InstNoOp(
                                name=f"I-wspill-{bb.name}-{uid}",
                                engine=inst.engine,
                                ins=[],
                                outs=[],
                                sync_info=bass_rust.SyncInfo(
                                    on_wait=head[i : i + max_waits], on_update=[]
                                ),
                            )
                        )
                        uid += 1
                        n += 1
                    si.on_wait = tail
                    changed = True
                out.append(inst)
            if changed:
                bb.instructions = out
    return n


def build_bass():
    nc = bass.Bass()
    ctxT_h = nc.declare_dram_parameter("ctxT", [BPC, D2, C_L], BF16, isOutput=False)
    q_h = nc.declare_dram_parameter("q", [BPC, Q_L, D2], BF16, isOutput=False)
    rp_h = nc.declare_dram_parameter("Rp", [BPC, D2, Q_L], BF16, isOutput=False)
    lng_h = nc.declare_dram_parameter("lng", [128, 2 * BPC], F32, isOutput=False)
    c2q_h = nc.declare_dram_parameter("c2q", [BPC, C_L, D2], BF16, isOutput=True)
    mx_h = nc.declare_dram_parameter("mx", [BPC, C_L], F32, isOutput=True)

    with TileContext(nc) as tc:
        with (
            tc.tile_pool(name="const", bufs=1) as cpool,
            tc.tile_pool(name="ld", bufs=1) as lpool,
            tc.tile_pool(name="ctx", bufs=2 * BPC) as xpool,
            tc.tile_pool(name="wen", bufs=2) as wen,
            tc.tile_pool(name="wm", bufs=2) as wm,
            tc.tile_pool(name="wcq", bufs=2) as wcq,
            tc.tile_pool(name="wr", bufs=2) as wr,
            tc.tile_pool(name="wmx", bufs=2) as wmx,
            tc.tile_pool(name="ps_s0", bufs=1, space="PSUM") as ps_s0,
            tc.tile_pool(name="ps_s1", bufs=2, space="PSUM") as ps_s1,
            tc.tile_pool(name="ps_cq", bufs=2, space="PSUM") as ps_cq,
            tc.tile_pool(name="ps_den", bufs=1, space="PSUM") as ps_den,
        ):
            # ---------------- loads (SP queue, in emission order) ------------
            rp_sb = lpool.tile([128, BPC * 2 * Q_L], BF16)
            q_sb = lpool.tile([128, BPC * 2 * D2], BF16)
            lng_sb = lpool.tile([128, 2 * BPC], F32)

            def load_rp(b):
                nc.sync.dma_start(
                    out=rp_sb[:, b * 512 : (b + 1) * 512].rearrange(
                        "p (dc j) -> p dc j", dc=2
                    ),
                    in_=rp_h[b].rearrange("(dc p) j -> p dc j", p=128),
                )

            def load_q(b):
                nc.sync.dma_start(
                    out=q_sb[:, b * 512 : (b + 1) * 512].rearrange(
                        "p (jc d) -> p jc d", jc=2
                    ),
                    in_=q_h[b].rearrange("(jc p) d -> p jc d", p=128),
                )

            ctx_tiles = {}

            def load_ctx(b, h):
                ct = xpool.tile([128, 2048], BF16, tag="c", name=f"c{b}{h}")
                nc.sync.dma_start(
                    out=ct[:].rearrange("p (dc c) -> p dc c", dc=2),
                    in_=ctxT_h[b, :, h * 1024 : (h + 1) * 1024].rearrange(
                        "(dc p) c -> p dc c", p=128
                    ),
                )
                ctx_tiles[(b, h)] = ct

            load_rp(0)
            load_ctx(0, 0)
            nc.sync.dma_start(out=lng_sb[:], in_=lng_h[:, :])
            load_q(0)
            load_ctx(0, 1)
            for b in range(1, BPC):
                load_rp(b)
                load_q(b)
            load_ctx(1, 0)
            # remaining ctx halves stream in during the slot loop
            late_loads = {1: (1, 1), 3: (2, 0), 5: (2, 1), 7: (3, 0), 9: (3, 1)}

            # ---------------- constants ----------------
            ones_col_b = cpool.tile([128, 1], BF16)
            nc.vector.memset(ones_col_b[:], 1.0)

            # ---------------- slot pipeline ----------------
            # slot s = (batch b, quarter qb); 512 context columns each.
            st = {}

            def stageA(s):
                b, qb = divmod(s, 4)
                h, sub = divmod(qb, 2)
                ct = ctx_tiles[(b, h)]
                ps = {}
                for jc in range(2):
                    pool = ps_s0 if jc == 0 else ps_s1
                    p = pool.tile([128, 512], F32, tag=f"s{jc}", name=f"s{jc}")
                    for dc in range(2):
                        nc.tensor.matmul(
                            p[:],
                            rp_sb[
                                :,
                                b * 512 + dc * 256 + jc * 128 : b * 512
                                + dc * 256
                                + (jc + 1) * 128,
                            ],
                            ct[:, dc * 1024 + sub * 512 : dc * 1024 + (sub + 1) * 512],
                            start=(dc == 0),
                            stop=(dc == 1),
                        )
                    ps[jc] = p
                # exps on ACT run while the PE works on the previous slot's B
                en = wen.tile([128, 1024], BF16, tag="en", name="en")
                for jc in range(2):
                    nc.scalar.activation(
                        en[:, jc * 512 : (jc + 1) * 512],
                        ps[jc][:],
                        AF.Exp,
                        bias=lng_sb[:, b * 2 + jc : b * 2 + jc + 1],
                    )
                st[s] = en

            def stageB(s):
                b, qb = divmod(s, 4)
                h, sub = divmod(qb, 2)
                en = st.pop(s)
                ct = ctx_tiles[(b, h)]

                # denominators first: 8 one-row matmuls feeding the DVE chain
                den = ps_den.tile([128, 4], F32, tag="den", name="den")
                for ch in range(4):
                    for jc in range(2):
                        nc.tensor.matmul(
                            den[:, ch : ch + 1],
                            en[:, jc * 512 + ch * 128 : jc * 512 + (ch + 1) * 128],
                            ones_col_b[:],
                            start=(jc == 0),
                            stop=(jc == 1),
                        )
                rden = wr.tile([128, 4], F32, tag="rden", name="rden")
                nc.vector.reciprocal(rden[:], den[:])

                # jc-premerge + partition-axis max while the PE does c2q
                enM = wm.tile([128, 512], BF16, tag="enM", name="enM")
                nc.vector.tensor_tensor(
                    out=enM[:], in0=en[:, 0:512], in1=en[:, 512:1024], op=OP.max
                )
                if qb == 0:
                    st[("mx", b)] = wmx.tile([1, C_L], F32, tag="mx", name=f"mx{b}")
                mx_sb = st[("mx", b)]
                nc.gpsimd.tensor_reduce(
                    out=mx_sb[0:1, qb * 512 : (qb + 1) * 512],
                    in_=enM[:],
                    axis=AX.C,
                    op=OP.max,
                )

                # c2q matmuls: 8 chunks of [128c, 256d], jc-chained
                cq = ps_cq.tile([128, 1024], F32, tag="cq", name="cq")
                for ch in range(4):
                    for jc in range(2):
                        nc.tensor.matmul(
                            cq[:, ch * 256 : (ch + 1) * 256],
                            en[:, jc * 512 + ch * 128 : jc * 512 + (ch + 1) * 128],
                            q_sb[:, (b * 2 + jc) * D2 : (b * 2 + jc + 1) * D2],
                            start=(jc == 0),
                            stop=(jc == 1),
                        )

                # normalize + f32->bf16 in one DVE broadcast-mult
                c2q_sb = wcq.tile([128, 1024], BF16, tag="cqs", name="cqs")
                nc.vector.tensor_mul(
                    c2q_sb[:].rearrange("p (t d) -> p t d", t=4),
                    cq[:].rearrange("p (t d) -> p t d", t=4),
                    rden[:]
                    .rearrange("p (t o) -> p t o", o=1)
                    .broadcast_to([128, 4, 256]),
                )
                nc.sync.dma_start(
                    out=c2q_h[b, qb * 512 : (qb + 1) * 512, :].rearrange(
                        "(t p) d -> p t d", p=128
                    ),
                    in_=c2q_sb[:].rearrange("p (t d) -> p t d", t=4),
                )
                if qb == 3:
                    nc.sync.dma_start(
                        out=mx_h[b : b + 1, :], in_=mx_sb[:]
                    )
                if s in late_loads:
                    load_ctx(*late_loads[s])

            for s in range(NSLOT):
                stageA(s)
                if s > 0:
                    stageB(s - 1)
            stageB(NSLOT - 1)

    _spill_excess_waits(nc)
    return nc


_NC_CACHE = None


def _get_nc():
    global _NC_CACHE
    if _NC_CACHE is None:
        _NC_CACHE = build_bass()
    return _NC_CACHE


def kernel(**inputs) -> np.ndarray:
    bf16 = ml_dtypes.bfloat16
    ctx = np.ascontiguousarray(np.asarray(inputs["context"], dtype=np.float32))
    cm = np.ascontiguousarray(np.asarray(inputs["context_mask"], dtype=np.float32))
    q = np.ascontiguousarray(np.asarray(inputs["query"], dtype=np.float32))
    qm = np.ascontiguousarray(np.asarray(inputs["query_mask"], dtype=np.float32))
    w = np.ascontiguousarray(np.asarray(inputs["W"], dtype=np.float32))
    w_c, w_q, w_cq = w[:D2], w[D2 : 2 * D2], w[2 * D2 :]

    ctx_bf = ctx.astype(bf16)
    # host-side prep: pre-transposed context, Rp = qT*w_cq + w_c, and the
    # per-j exp bias lng = q.w_q + ln(qm) in partition-major layout
    ctxT = np.ascontiguousarray(ctx_bf.transpose(0, 2, 1))          # [B,D2,C_L]
    rp = np.ascontiguousarray(
        (q.transpose(0, 2, 1) * w_cq[None, :, None] + w_c[None, :, None]).astype(bf16)
    )                                                               # [B,D2,Q_L]
    q_bf = q.astype(bf16)
    lng = np.einsum("bjd,d->bj", q, w_q) + np.log(qm + 1e-38)       # [B,Q_L]

    in_maps = []
    for core in range(N_CORES):
        lo, hi = core * BPC, (core + 1) * BPC
        lng_c = lng[lo:hi].reshape(BPC, 2, 128).transpose(2, 0, 1).reshape(128, 2 * BPC)
        in_maps.append(
            {
                "ctxT": ctxT[lo:hi],
                "q": q_bf[lo:hi],
                "Rp": rp[lo:hi],
                "lng": np.ascontiguousarray(lng_c),
            }
        )

    nc = _get_nc()
    res = run_bass_kernel_spmd(nc, in_maps, list(range(N_CORES)))

    c2q = np.empty((B, C_L, D2), dtype=np.float32)
    mx = np.empty((B, C_L), dtype=np.float32)
    for i in range(N_CORES):
        lo, hi = i * BPC, (i + 1) * BPC
        c2q[lo:hi] = np.asarray(res.results[i]["c2q"]).astype(np.float32)
        mx[lo:hi] = np.asarray(res.results[i]["mx"])

    # host-side Q2C: s_max = ln(mx) reproduces masked_S.max(-1) exactly for
    # rows with >=1 valid j (en of masked j is 0 and never the max)
    s_max = np.log(np.maximum(mx, 1e-300))
    v = s_max * cm
    e = np.exp(v - v.max(axis=-1, keepdims=True))
    sm = e / e.sum(axis=-1, keepdims=True)
    attn = sm * cm
    attn = attn / (attn.sum(axis=-1, keepdims=True) + EPS)
    q2c = np.einsum("bc,bcd->bd", attn, ctx)                        # [B,D2]

    out = np.empty((B, C_L, 4 * D2), dtype=np.float32)
    out[:, :, 0:D2] = ctx
    out[:, :, D2 : 2 * D2] = c2q
    out[:, :, 2 * D2 : 3 * D2] = ctx * c2q
    out[:, :, 3 * D2 :] = ctx * q2c[:, None, :]
    return out


# revision 13
# speedup vs baseline: 1.7670x; 1.2021x over previous
"""BiAttention (BiDAF-style) Trainium2 kernel, v3: fp8 DoubleRow similarity +
host reconstruction of everything derivable from small device outputs.

G = [c, c2q, c*c2q, c*q2c].  The host already holds c, so the device only
produces
  c2q[b,c,:]  (C2Q attention-weighted query rows, bf16, 4 MiB/core) and
  mx[b,c]     (max_j of the masked exp similarities, f32, 32 KB/core),
from which the host derives s_max = ln(mx), the Q2C softmax, q2c, and the
three elementwise output columns in f32 (col0 is exactly c).

Device math per batch (masks exact {0,1}):
  Rp[d,j]  = fp8e4(16*(qT[d,j]*w_cq[d] + w_c[d]))  (host-computed; x16 keeps
                                                    the ~N(0,0.07) values out
                                                    of fp8 subnormals)
  enT[j,c] = exp(S16[j,c]/16 + lng[j])             (ACT exp, scale=1/16,
            S16 = sum_d Rp[d,j] ctx8[d,c]           bias=lng; ONE DoubleRow
                                                    fp8 matmul per jc: both
                                                    128-row d-tiles contract
                                                    in a single pass)
  den[c]   = sum_j enT[j,c]                        (ones-column matmuls)
  c2q      = (enT^T @ q) * (1/den)                 (bf16 PE + DVE bcast-mult)
  mx[c]    = max_j enT[j,c]                        (jc-premerge on DVE/Pool +
                                                    gpsimd partition reduce)

Only the similarity matmul runs in fp8 (it feeds a softmax, where ~0.1 logit
noise averages out); en/q/c2q stay bf16, keeping output error well below the
2e-2 gate.  The context arrives host-pre-transposed as ctx8[b, d, c] so the
PE does no transposes; Rp and q arrive pre-laid in their exact SBUF layouts
so each is one big contiguous DMA.  Per 512-column slot the PE streams ~1080
ns, ACT 1224 (exps), DVE ~1400 (recip + normalize + 1/4 of premerges), Pool
~1400 (partition reduce + 3/4 of premerges), DMA ~1100 — a balanced ~1.4
us/slot pipeline.  The final slot reorders its tail (premerge/reduce before
c2q, split normalize + split store) to shorten the post-PE drain.
"""

import numpy as np
import ml_dtypes

import bass_rust
import concourse.bass as bass
import concourse.mybir as mybir
from concourse.tile import TileContext
from concourse.bass_utils import run_bass_kernel_spmd

F32 = mybir.dt.float32
BF16 = mybir.dt.bfloat16
FP8 = mybir.dt.float8e4
AF = mybir.ActivationFunctionType
OP = mybir.AluOpType
AX = mybir.AxisListType
PM = mybir.MatmulPerfMode

N_CORES = 8
B, C_L, Q_L, D2 = 32, 2048, 256, 256
BPC = B // N_CORES          # batches per core
NSLOT = BPC * 4             # 512-column slots per core
EPS = 1e-13
RP_SCALE = 16.0


def _spill_excess_waits(nc, max_waits: int = 1) -> int:
    """The installed walrus rejects >1 sync wait per instruction. Hoist excess
    waits onto same-engine InstNoOp carriers inserted just before."""
    n = 0
    uid = 0
    for f in nc.m.functions:
        for bb in f.blocks:
            out = []
            changed = False
            for inst in bb.instructions:
                si = inst.sync_info
                waits = list(si.on_wait) if si is not None and si.on_wait else []
                if len(waits) > max_waits:
                    head, tail = waits[:-max_waits], waits[-max_waits:]
                    for i in range(0, len(head), max_waits):
                        out.append(
                            mybir.InstNoOp(
                                name=f"I-wspill-{bb.name}-{uid}",
                                engine=inst.engine,
                                ins=[],
                                outs=[],
                                sync_info=bass_rust.SyncInfo(
                                    on_wait=head[i : i + max_waits], on_update=[]
                                ),
                            )
                        )
                        uid += 1
                        n += 1
                    si.on_wait = tail
                    changed = True
                out.append(inst)
            if changed:
                bb.instructions = out
    return n


def build_bass():
    nc = bass.Bass()
    ctx_h = nc.declare_dram_parameter("ctx8", [BPC, D2, C_L], FP8, isOutput=False)
    q_h = nc.declare_dram_parameter("q", [128, BPC * 2 * D2], BF16, isOutput=False)
    rp_h = nc.declare_dram_parameter("Rp", [128, BPC * 2 * Q_L], FP8, isOutput=False)
    lng_h = nc.declare_dram_parameter("lng", [128, 2 * BPC], F32, isOutput=False)
    c2q_h = nc.declare_dram_parameter("c2q", [BPC, C_L, D2], BF16, isOutput=True)
    # per batch: 512 jc-merged maxes for quarter 0, then 3 x 1024 per-jc
    # maxes for quarters 1-3 (host merges the jc pair)
    mx_h = nc.declare_dram_parameter("mx", [BPC, 3584], F32, isOutput=True)

    with TileContext(nc) as tc:
        with (
            tc.tile_pool(name="const", bufs=1) as cpool,
            tc.tile_pool(name="ld", bufs=1) as lpool,
            tc.tile_pool(name="ctx", bufs=NSLOT) as xpool,
            tc.tile_pool(name="wen", bufs=4) as wen,
            tc.tile_pool(name="wm", bufs=3) as wm,
            tc.tile_pool(name="wcq", bufs=3) as wcq,
            tc.tile_pool(name="wr", bufs=3) as wr,
            tc.tile_pool(name="wmx", bufs=2) as wmx,
            tc.tile_pool(name="ps_s0", bufs=1, space="PSUM") as ps_s0,
            tc.tile_pool(name="ps_s1", bufs=2, space="PSUM") as ps_s1,
            tc.tile_pool(name="ps_cq", bufs=2, space="PSUM") as ps_cq,
            tc.tile_pool(name="ps_den", bufs=1, space="PSUM") as ps_den,
        ):
            # ---------------- loads (SP queue, in emission order) ------------
            # Rp and q come from DRAM already in SBUF layout: one contiguous
            # DMA each, no sub-512B elements anywhere.
            rp_sb = lpool.tile([128, BPC * 2 * Q_L], FP8)
            q_sb = lpool.tile([128, BPC * 2 * D2], BF16)
            lng_sb = lpool.tile([128, 2 * BPC], F32)
            ctx_tiles = {}

            def load_ctx(s):
                b, qb = divmod(s, 4)
                ct = xpool.tile([128, 1024], FP8, tag="c", name=f"c{s}")
                nc.sync.dma_start(
                    out=ct[:].rearrange("p (dc c) -> p dc c", dc=2),
                    in_=ctx_h[b, :, qb * 512 : (qb + 1) * 512].rearrange(
                        "(dc p) c -> p dc c", p=128
                    ),
                )
                ctx_tiles[s] = ct

            load_ctx(0)
            nc.sync.dma_start(out=rp_sb[:], in_=rp_h[:, :])
            nc.sync.dma_start(out=lng_sb[:], in_=lng_h[:, :])
            load_ctx(1)
            nc.sync.dma_start(out=q_sb[:], in_=q_h[:, :])
            load_ctx(2)
            load_ctx(3)

            # ---------------- constants ----------------
            ones_col_b = cpool.tile([128, 1], BF16)
            nc.vector.memset(ones_col_b[:], 1.0)

            # ---------------- slot pipeline ----------------
            # slot s = (batch b, quarter qb); 512 context columns each.
            st = {}

            def stageA(s):
                b, qb = divmod(s, 4)
                ct3 = ctx_tiles[s][:].rearrange("p (dc c) -> p dc c", dc=2)
                rp3 = rp_sb[:, b * 512 : (b + 1) * 512].rearrange(
                    "p (dc j) -> p dc j", dc=2
                )
                ps = {}
                for jc in range(2):
                    pool = ps_s0 if jc == 0 else ps_s1
                    p = pool.tile([128, 512], F32, tag=f"s{jc}", name=f"s{jc}")
                    nc.tensor.matmul(
                        p[:],
                        rp3[:, :, jc * 128 : (jc + 1) * 128],
                        ct3,
                        start=True,
                        stop=True,
                        perf_mode=PM.DoubleRow,
                    )
                    ps[jc] = p
                # exps on ACT run while the PE works on the previous slot's B;
                # scale undoes the x16 fp8 pre-scale of Rp
                en = wen.tile([128, 1024], BF16, tag="en", name="en")
                for jc in range(2):
                    nc.scalar.activation(
                        en[:, jc * 512 : (jc + 1) * 512],
                        ps[jc][:],
                        AF.Exp,
                        bias=lng_sb[:, b * 2 + jc : b * 2 + jc + 1],
                        scale=1.0 / RP_SCALE,
                    )
                st[s] = en

            def stageB(s):
                b, qb = divmod(s, 4)
                en = st.pop(s)
                last = s == NSLOT - 1

                # denominators first: 8 one-row matmuls feeding the DVE chain
                den = ps_den.tile([128, 4], F32, tag="den", name="den")
                for ch in range(4):
                    for jc in range(2):
                        nc.tensor.matmul(
                            den[:, ch : ch + 1],
                            en[:, jc * 512 + ch * 128 : jc * 512 + (ch + 1) * 128],
                            ones_col_b[:],
                            start=(jc == 0),
                            stop=(jc == 1),
                        )
                rden = wr.tile([128, 4], F32, tag="rden", name="rden")
                nc.vector.reciprocal(rden[:], den[:])

                # partition-axis max.  Quarter 0: DVE jc-premerge + narrow
                # gpsimd reduce; quarters 1-3: one wide gpsimd reduce over
                # both jc column groups (host merges the jc pair).  This
                # balances DVE (~1.40us/slot) against Pool (~1.34us/slot).
                def mx_work():
                    if qb == 0:
                        st[("mx", b)] = wmx.tile([1, 3584], F32, tag="mx", name=f"mx{b}")
                        enM = wm.tile([128, 512], BF16, tag="enM", name="enM")
                        nc.vector.tensor_tensor(
                            out=enM[:], in0=en[:, 0:512], in1=en[:, 512:1024],
                            op=OP.max,
                        )
                        nc.gpsimd.tensor_reduce(
                            out=st[("mx", b)][0:1, 0:512],
                            in_=enM[:],
                            axis=AX.C,
                            op=OP.max,
                        )
                    else:
                        nc.gpsimd.tensor_reduce(
                            out=st[("mx", b)][0:1, qb * 1024 - 512 : (qb + 1) * 1024 - 512],
                            in_=en[:],
                            axis=AX.C,
                            op=OP.max,
                        )

                if last:
                    mx_work()  # before c2q so the reduce drains during it

                # c2q matmuls: 8 chunks of [128c, 256d], jc-chained
                cq = ps_cq.tile([128, 1024], F32, tag="cq", name="cq")
                for ch in range(4):
                    for jc in range(2):
                        nc.tensor.matmul(
                            cq[:, ch * 256 : (ch + 1) * 256],
                            en[:, jc * 512 + ch * 128 : jc * 512 + (ch + 1) * 128],
                            q_sb[:, (b * 2 + jc) * D2 : (b * 2 + jc + 1) * D2],
                            start=(jc == 0),
                            stop=(jc == 1),
                        )

                # normalize + f32->bf16 via DVE broadcast-mult; the last slot
                # splits it (and the store) in half so HBM writes start while
                # the second half is still normalizing
                c2q_sb = wcq.tile([128, 1024], BF16, tag="cqs", name="cqs")
                for lo_t, hi_t in ((0, 2), (2, 4)) if last else ((0, 4),):
                    t = hi_t - lo_t
                    sl = slice(lo_t * 256, hi_t * 256)
                    nc.vector.tensor_mul(
                        c2q_sb[:, sl].rearrange("p (t d) -> p t d", t=t),
                        cq[:, sl].rearrange("p (t d) -> p t d", t=t),
                        rden[:, lo_t:hi_t]
                        .rearrange("p (t o) -> p t o", o=1)
                        .broadcast_to([128, t, 256]),
                    )
                    nc.sync.dma_start(
                        out=c2q_h[
                            b, qb * 512 + lo_t * 128 : qb * 512 + hi_t * 128, :
                        ].rearrange("(t p) d -> p t d", p=128),
                        in_=c2q_sb[:, sl].rearrange("p (t d) -> p t d", t=t),
                    )

                if not last:
                    mx_work()
                if qb == 3:
                    nc.sync.dma_start(
                        out=mx_h[b : b + 1, :], in_=st.pop(("mx", b))[:]
                    )
                if s + 4 < NSLOT:
                    load_ctx(s + 4)

            for s in range(NSLOT):
                stageA(s)
                if s > 0:
                    stageB(s - 1)
            stageB(NSLOT - 1)

    _spill_excess_waits(nc)
    return nc


_NC_CACHE = None


def _get_nc():
    global _NC_CACHE
    if _NC_CACHE is None:
        _NC_CACHE = build_bass()
    return _NC_CACHE


def kernel(**inputs) -> np.ndarray:
    bf16 = ml_dtypes.bfloat16
    fp8 = ml_dtypes.float8_e4m3fn
    ctx = np.ascontiguousarray(np.asarray(inputs["context"], dtype=np.float32))
    cm = np.ascontiguousarray(np.asarray(inputs["context_mask"], dtype=np.float32))
    q = np.ascontiguousarray(np.asarray(inputs["query"], dtype=np.float32))
    qm = np.ascontiguousarray(np.asarray(inputs["query_mask"], dtype=np.float32))
    w = np.ascontiguousarray(np.asarray(inputs["W"], dtype=np.float32))
    w_c, w_q, w_cq = w[:D2], w[D2 : 2 * D2], w[2 * D2 :]

    # host-side prep: pre-transposed fp8 context; Rp = 16*(qT*w_cq + w_c) in
    # fp8; q in bf16; exp bias lng = q.w_q + ln(qm).  Rp/q/lng are laid out
    # exactly as their SBUF tiles ([partition, free]) for single-DMA loads.
    ctx8 = np.ascontiguousarray(
        np.clip(ctx, -440.0, 440.0).transpose(0, 2, 1).astype(fp8)
    )                                                               # [B,D2,C_L]
    rp = RP_SCALE * (q.transpose(0, 2, 1) * w_cq[None, :, None] + w_c[None, :, None])
    rp8 = np.clip(rp, -440.0, 440.0).astype(fp8)                    # [B,D2,Q_L]
    q_bf = q.astype(bf16)
    lng = np.einsum("bjd,d->bj", q, w_q) + np.log(qm + 1e-38)       # [B,Q_L]

    in_maps = []
    for core in range(N_CORES):
        lo, hi = core * BPC, (core + 1) * BPC
        # Rp: [BPC,D2,Q_L] -> [128, (b, dc, j)] with d = dc*128 + p
        rp_c = (
            rp8[lo:hi]
            .reshape(BPC, 2, 128, Q_L)
            .transpose(2, 0, 1, 3)
            .reshape(128, BPC * 2 * Q_L)
        )
        # q: [BPC,Q_L,D2] -> [128, (b, jc, d)] with j = jc*128 + p
        q_c = (
            q_bf[lo:hi]
            .reshape(BPC, 2, 128, D2)
            .transpose(2, 0, 1, 3)
            .reshape(128, BPC * 2 * D2)
        )
        lng_c = lng[lo:hi].reshape(BPC, 2, 128).transpose(2, 0, 1).reshape(128, 2 * BPC)
        in_maps.append(
            {
                "ctx8": ctx8[lo:hi],
                "q": np.ascontiguousarray(q_c),
                "Rp": np.ascontiguousarray(rp_c),
                "lng": np.ascontiguousarray(lng_c),
            }
        )

    nc = _get_nc()
    res = run_bass_kernel_spmd(nc, in_maps, list(range(N_CORES)))

    c2q = np.empty((B, C_L, D2), dtype=np.float32)
    mx = np.empty((B, C_L), dtype=np.float32)
    for i in range(N_CORES):
        lo, hi = i * BPC, (i + 1) * BPC
        c2q[lo:hi] = np.asarray(res.results[i]["c2q"]).astype(np.float32)
        mxd = np.asarray(res.results[i]["mx"])      # [BPC, 3584]
        mx[lo:hi, 0:512] = mxd[:, 0:512]            # quarter 0: jc-merged
        mx[lo:hi, 512:] = mxd[:, 512:].reshape(BPC, 3, 2, 512).max(axis=2).reshape(
            BPC, 1536
        )

    # host-side Q2C: s_max = ln(mx) reproduces masked_S.max(-1) exactly for
    # rows with >=1 valid j (en of masked j is 0 and never the max)
    s_max = np.log(np.maximum(mx, 1e-300))
    v = s_max * cm
    e = np.exp(v - v.max(axis=-1, keepdims=True))
    sm = e / e.sum(axis=-1, keepdims=True)
    attn = sm * cm
    attn = attn / (attn.sum(axis=-1, keepdims=True) + EPS)
    q2c = np.einsum("bc,bcd->bd", attn, ctx)                        # [B,D2]

    out = np.empty((B, C_L, 4 * D2), dtype=np.float32)
    out[:, :, 0:D2] = ctx
    out[:, :, D2 : 2 * D2] = c2q
    out[:, :, 2 * D2 : 3 * D2] = ctx * c2q
    out[:, :, 3 * D2 :] = ctx * q2c[:, None, :]
    return out
